# revision 15
# baseline (speedup 1.0000x reference)
"""GAT (2-layer graph attention network) on 8 Trainium2 NeuronCores.

Strategy (per spec sharding hint): shard the node dim N=4096 across 8 cores
(512 rows each). Each core computes its [512, 4096] slice of each attention
matrix; row-wise softmax is local. h (layer-1 features) and h_out (layer-2
input) are all-gathered across cores via AllGather collectives.

Key kernel structure (per core), all in "transposed" layout [j-partition,
i-free] so the att @ h contraction lands on the PE partition dim:

  e[i,j] = leakyrelu(s_src[i] + s_dst[j]) is rank-1 before the nonlinearity:
  s_src/s_dst are tiny per-node scalars, so no QK matmul is needed.
  The mask is folded additively pre-lrelu: t = s_src + s_dst + (adj-1)*1500
  => exp(lrelu(t)) == adj * exp(lrelu(e)) exactly in f32 (underflow to 0).

  Per (head, j-block) tile [128j, 512i]:
    STT-1 (DVE scalar_tensor_tensor): t = (src_bcast + s_dst[jb]) + mask_bias
    lrelu: DVE STT (t*0.2 max t) for 3/4 tiles, ACT Prelu for 1/4 (balance)
    ACT Exp (batched over 4 jb)  -> p tile (bf16)
    PE matmul: psum[65, 512] += [h[jb] | ones].T @ p   (ones row = softmax
    denominator, accumulated over all 32 j-blocks)

  Then per head: normalize by the denominator row, ELU, assemble x_catT;
  h_out = x_catT @ W_out; AllGather h_out; layer-2 attention (same scheme,
  row-form output) and log_softmax.
"""
import sys
import time

sys.path.insert(0, "/opt/trn_rl_repo")

import numpy as np
import ml_dtypes

import concourse.bass as bass
import concourse.bacc as bacc
import concourse.tile as tile
from concourse import mybir
from concourse.bass_utils import run_bass_kernel_spmd
from concourse.masks import make_identity

dt = mybir.dt
BF = ml_dtypes.bfloat16

N, NFEAT, NHID, NHEAD, NCLASS = 4096, 1024, 64, 8, 32
NCORES = 8
R = N // NCORES          # 512 rows per core
NJB = N // 128           # 32 j-blocks
KCH = NFEAT // 128       # 8 full K chunks for x@W (+1 for bias row)
MASK_BIG = 1500.0
ALPHA = 0.2

_cached = {}


def _build_program():
    nc = bacc.Bacc("TRN2", target_bir_lowering=False, debug=False,
                   enable_asserts=False, num_devices=NCORES)

    xT = nc.dram_tensor("xT", [NFEAT + 1, R], dt.bfloat16, kind="ExternalInput").ap()
    wh = nc.dram_tensor("wh", [NHEAD, NFEAT + 1, NHID], dt.bfloat16, kind="ExternalInput").ap()
    adjT = nc.dram_tensor("adjT", [N, R], dt.bfloat16, kind="ExternalInput").ap()
    aT = nc.dram_tensor("aT", [NHEAD, NHID, 2], dt.bfloat16, kind="ExternalInput").ap()
    wo = nc.dram_tensor("wo", [NHEAD * NHID + 1, NCLASS], dt.bfloat16, kind="ExternalInput").ap()
    ao = nc.dram_tensor("ao", [NCLASS, 2], dt.float32, kind="ExternalInput").ap()
    out = nc.dram_tensor("out", [R, NCLASS], dt.float32, kind="ExternalOutput").ap()

    with tile.TileContext(nc, num_cores=NCORES) as tc:
        _emit(nc, tc, xT, wh, adjT, aT, wo, ao, out)
    nc.compile()
    return nc


def _emit(nc, tc, xT, wh, adjT, aT, wo, ao, out):
    from contextlib import ExitStack
    f32, bf16 = dt.float32, dt.bfloat16
    AF = mybir.ActivationFunctionType
    OP = mybir.AluOpType
    AG = "AllGather"

    cst_ctx = ExitStack()
    cst = cst_ctx.enter_context(tc.tile_pool(name="cst", bufs=1))
    dram = cst_ctx.enter_context(tc.tile_pool(name="dram", bufs=1, space="DRAM"))

    # ---- collective buffers ----
    cc_s_in = dram.tile([2 * NHEAD, R], f32)
    cc_s_out = dram.tile([NCORES, 2 * NHEAD, R], f32, addr_space="Shared")
    cc_h_in = [dram.tile([R, NHID], bf16, name=f"cc_h_in{h}") for h in range(NHEAD)]
    cc_h_out = [dram.tile([NCORES, R, NHID], bf16, addr_space="Shared",
                          name=f"cc_h_out{h}") for h in range(NHEAD)]
    cc_ho_in = dram.tile([R, NCLASS], bf16)
    cc_ho_out = dram.tile([NCORES, R, NCLASS], bf16, addr_space="Shared")
    cc_s2_in = dram.tile([2, R], f32)
    cc_s2_out = dram.tile([NCORES, 2, R], f32, addr_space="Shared")
    groups = [list(range(NCORES))]

    # ---- persistent SBUF ----
    mT = cst.tile([128, NJB, R], bf16)            # raw 0/1 mask, transposed
    nc.scalar.dma_start(
        out=mT,
        in_=adjT.rearrange("(jb p) i -> p jb i", p=128))

    h_rhs = [cst.tile([128, NJB, NHID + 1], bf16, name=f"h_rhs{h}")
             for h in range(NHEAD)]
    for h in range(NHEAD):
        nc.vector.memset(h_rhs[h][:, :, NHID:NHID + 1], 1.0)

    src_bc = [cst.tile([128, R], bf16, name=f"src_bc{h}") for h in range(NHEAD)]
    src02_bc = [cst.tile([128, R], bf16, name=f"src02_bc{h}") for h in range(NHEAD)]
    sdst = cst.tile([128, NHEAD, NJB], f32)
    sdst02 = cst.tile([128, NHEAD, NJB], f32)
    ident64 = cst.tile([64, 64], bf16)
    make_identity(nc, ident64)
    ident128 = cst.tile([128, 128], f32)
    make_identity(nc, ident128)
    ones64 = cst.tile([1, 64], f32)
    nc.vector.memset(ones64, 1.0)
    ones_row = cst.tile([1, R], bf16)
    nc.vector.memset(ones_row, 1.0)
    xcatT = [cst.tile([128, R], bf16, name=f"xcatT{k}") for k in range(4)]
    h2_rhs = cst.tile([128, NJB, NCLASS + 1], bf16)
    nc.vector.memset(h2_rhs[:, :, NCLASS:NCLASS + 1], 1.0)
    src2_bc = cst.tile([128, R], bf16)
    src202_bc = cst.tile([128, R], bf16)
    s2dst = cst.tile([128, NJB], f32)
    s2dst02 = cst.tile([128, NJB], f32)

    # =================== Stage A: h = x @ W per head, s vectors ============
    stA = ExitStack()
    sa = stA.enter_context(tc.tile_pool(name="sa", bufs=1))
    psA = stA.enter_context(tc.tile_pool(name="psA", bufs=1, space="PSUM"))

    xT_sb = sa.tile([128, KCH + 1, R], bf16)
    nc.sync.dma_start(out=xT_sb[:, 0:KCH, :],
                      in_=xT[0:NFEAT, :].rearrange("(k p) i -> p k i", p=128))
    nc.sync.dma_start(out=xT_sb[0:1, KCH, :], in_=xT[NFEAT:NFEAT + 1, :])
    wh_sb = sa.tile([128, NHEAD, KCH + 1, NHID], bf16)
    for h in range(NHEAD):
        nc.scalar.dma_start(out=wh_sb[:, h, 0:KCH, :],
                            in_=wh[h, 0:NFEAT, :].rearrange("(k p) o -> p k o", p=128))
        nc.scalar.dma_start(out=wh_sb[0:1, h, KCH, :], in_=wh[h, NFEAT:NFEAT + 1, :])
    aT_sb = sa.tile([64, NHEAD, 2], bf16)
    nc.sync.dma_start(out=aT_sb, in_=aT.rearrange("h o k -> o h k"))

    hT_sb = sa.tile([64, NHEAD, R], bf16)
    for h in range(NHEAD):
        ps_hT = psA.tile([64, R], f32, tag="hT", bufs=2)
        for k in range(KCH + 1):
            kp = 128 if k < KCH else 1
            nc.tensor.matmul(ps_hT, lhsT=wh_sb[0:kp, h, k, :],
                             rhs=xT_sb[0:kp, k, :],
                             start=(k == 0), stop=(k == KCH))
        nc.scalar.copy(out=hT_sb[:, h, :], in_=ps_hT)
        ps_s1 = psA.tile([2, R], f32, tag="s1", bufs=2)
        nc.tensor.matmul(ps_s1, lhsT=aT_sb[:, h, :], rhs=hT_sb[:, h, :],
                         start=True, stop=True)
        s1_sb = sa.tile([2, R], f32, tag="s1sb", bufs=2)
        nc.vector.tensor_copy(out=s1_sb, in_=ps_s1)
        nc.sync.dma_start(out=cc_s_in[2 * h:2 * h + 2, :], in_=s1_sb)
        h_row4 = sa.tile([128, 4, 64], bf16, tag="hrow", bufs=2)
        for tb in range(4):
            ps_tr = psA.tile([128, 64], bf16, tag="tr", bufs=2)
            nc.tensor.transpose(ps_tr, hT_sb[:, h, tb * 128:(tb + 1) * 128], ident64)
            nc.vector.tensor_copy(out=h_row4[:, tb, :], in_=ps_tr)
        nc.sync.dma_start(out=cc_h_in[h].rearrange("(l p) o -> p l o", p=128),
                          in_=h_row4)

    nc.gpsimd.collective_compute(AG, mybir.AluOpType.bypass, replica_groups=groups,
                                 ins=[cc_s_in[:]], outs=[cc_s_out[:]])
    for h in range(NHEAD):
        nc.gpsimd.collective_compute(AG, mybir.AluOpType.bypass, replica_groups=groups,
                                     ins=[cc_h_in[h][:]], outs=[cc_h_out[h][:]])

    # src_bc: broadcast own s_src rows (local, no gather needed)
    for h in range(NHEAD):
        row = cc_s_in[2 * h:2 * h + 1, :]
        bc_ap = bass.AP(tensor=row.tensor, offset=row.offset,
                        ap=[[0, 128]] + row.ap[1:])
        srcf = sa.tile([128, R], f32, tag="srcf", bufs=2)
        nc.sync.dma_start(out=srcf, in_=bc_ap)
        nc.vector.tensor_copy(out=src_bc[h], in_=srcf)
        nc.vector.tensor_scalar(out=src02_bc[h], in0=srcf, scalar1=ALPHA,
                                scalar2=None, op0=OP.mult)

    # sdst tiles first (small, unblock the attend pipeline), then h_rhs
    for h in range(NHEAD):
        for core in range(NCORES):
            nc.sync.dma_start(
                out=sdst[:, h, core * 4:(core + 1) * 4],
                in_=cc_s_out[core, 2 * h + 1, :].rearrange("(l p) -> p l", p=128))
    sdst2d = sdst.rearrange("p a b -> p (a b)")
    sdst02_2d = sdst02.rearrange("p a b -> p (a b)")
    nc.vector.tensor_scalar(out=sdst02_2d, in0=sdst2d, scalar1=ALPHA,
                            scalar2=None, op0=OP.mult)
    for h in range(NHEAD):
        for core in range(NCORES):
            eng = (nc.scalar, nc.sync)[(h * NCORES + core) % 2]
            eng.dma_start(
                out=h_rhs[h][:, core * 4:(core + 1) * 4, 0:NHID],
                in_=cc_h_out[h][core, :, :].rearrange("(l p) o -> p l o", p=128))

    stA.close()

    # =================== Stage B: layer-1 attention ========================
    stB = ExitStack()
    sb_ = stB.enter_context(tc.tile_pool(name="sb", bufs=1))
    psB_ctx = ExitStack()
    psB = psB_ctx.enter_context(tc.tile_pool(name="psB", bufs=1, space="PSUM"))

    def attend_tiles(src_tile, src02_tile, sdst_ap_fn, sdst02_ap_fn, q_sink, goff):
        """Emit the 32 j-block elementwise chain; call q_sink(jb, q_slice)."""
        for jbg in range(NJB // 4):
            route_act = ((goff + jbg) % 2) == 0
            eL = sb_.tile([128, 4, R], bf16, tag="eL", bufs=3)
            if route_act:
                for j4 in range(4):
                    jb = jbg * 4 + j4
                    nc.scalar.activation(out=eL[:, j4, :], in_=src_tile,
                                         func=AF.Prelu, bias=sdst_ap_fn(jb),
                                         scale=1.0, alpha=ALPHA)
            else:
                t4 = sb_.tile([128, 4, R], bf16, tag="t4", bufs=2)
                e5 = sb_.tile([128, 4, R], bf16, tag="e5", bufs=2)
                for j4 in range(4):
                    jb = jbg * 4 + j4
                    nc.vector.tensor_scalar(out=t4[:, j4, :], in0=src_tile,
                                            scalar1=sdst_ap_fn(jb), scalar2=None,
                                            op0=OP.add)
                    nc.vector.tensor_scalar(out=e5[:, j4, :], in0=src02_tile,
                                            scalar1=sdst02_ap_fn(jb), scalar2=None,
                                            op0=OP.add)
                nc.vector.tensor_tensor(out=eL, in0=t4, in1=e5, op=OP.max)
            q = sb_.tile([128, 4, R], bf16, tag="q", bufs=3)
            nc.scalar.activation(out=q, in_=eL, func=AF.Exp)
            nc.vector.tensor_tensor(out=q, in0=q,
                                    in1=mT[:, jbg * 4:(jbg + 1) * 4, :], op=OP.mult)
            for j4 in range(4):
                q_sink(jbg * 4 + j4, q[:, j4, :])

    for h in range(NHEAD):
        ps_att = psB.tile([NHID + 1, R], f32, tag="att", bufs=2)

        def sink(jb, qs, ps_att=ps_att, h=h):
            nc.tensor.matmul(ps_att, lhsT=h_rhs[h][:, jb, :], rhs=qs,
                             start=(jb == 0), stop=(jb == NJB - 1))

        attend_tiles(src_bc[h], src02_bc[h],
                     lambda jb, h=h: sdst[:, h, jb:jb + 1],
                     lambda jb, h=h: sdst02[:, h, jb:jb + 1], sink, goff=h * 8)

        # normalize + ELU -> x_catT
        dinv = sb_.tile([1, R], f32, tag="dinv", bufs=2)
        nc.vector.reciprocal(out=dinv, in_=ps_att[NHID:NHID + 1, :])
        ps_bc = psB.tile([64, R], f32, tag="bc", bufs=2)
        nc.tensor.matmul(ps_bc, lhsT=ones64, rhs=dinv, start=True, stop=True)
        att_sb = sb_.tile([64, R], f32, tag="attsb", bufs=2)
        nc.scalar.copy(out=att_sb, in_=ps_att[0:NHID, :])
        nc.vector.tensor_tensor(out=att_sb, in0=att_sb, in1=ps_bc, op=OP.mult)
        attn = att_sb
        neg = sb_.tile([64, R], f32, tag="neg", bufs=2)
        nc.vector.tensor_scalar(out=neg, in0=attn, scalar1=0.0, scalar2=None,
                                op0=OP.min)
        q2 = sb_.tile([64, R], f32, tag="q2", bufs=2)
        nc.scalar.activation(out=q2, in_=neg, func=AF.Exp)
        pos = sb_.tile([64, R], f32, tag="pos", bufs=2)
        nc.vector.tensor_scalar(out=pos, in0=attn, scalar1=0.0, scalar2=-1.0,
                                op0=OP.max, op1=OP.add)
        nc.vector.tensor_tensor(out=xcatT[h // 2][64 * (h % 2):64 * (h % 2) + 64, :],
                                in0=pos, in1=q2, op=OP.add)

    psB_ctx.close()

    # =================== Stage C: h_out = x_cat @ W_out, s2, gathers =======
    stC = ExitStack()
    sc = stC.enter_context(tc.tile_pool(name="sc", bufs=1))
    psC_ctx = ExitStack()
    psC = psC_ctx.enter_context(tc.tile_pool(name="psC", bufs=1, space="PSUM"))

    wo_sb = sc.tile([128, 5, NCLASS], bf16)
    nc.sync.dma_start(out=wo_sb[:, 0:4, :],
                      in_=wo[0:NHEAD * NHID, :].rearrange("(k p) c -> p k c", p=128))
    nc.sync.dma_start(out=wo_sb[0:1, 4, :], in_=wo[NHEAD * NHID:NHEAD * NHID + 1, :])
    ao_sb = sc.tile([32, 2], f32)
    nc.sync.dma_start(out=ao_sb, in_=ao)

    ps_ho = psC.tile([128, 4, NCLASS], f32)
    for ib in range(4):
        isl = slice(ib * 128, (ib + 1) * 128)
        for k in range(5):
            if k < 4:
                nc.tensor.matmul(ps_ho[:, ib, :], lhsT=xcatT[k][:, isl],
                                 rhs=wo_sb[:, k, :], start=(k == 0), stop=False)
            else:
                nc.tensor.matmul(ps_ho[:, ib, :], lhsT=ones_row[:, isl],
                                 rhs=wo_sb[0:1, 4, :], start=False, stop=True)
    h_out_sb = sc.tile([128, 4, NCLASS], f32)
    nc.scalar.copy(out=h_out_sb, in_=ps_ho)
    h_out_bf = sc.tile([128, 4, NCLASS], bf16)
    nc.vector.tensor_copy(out=h_out_bf, in_=h_out_sb)
    for ib in range(4):
        nc.sync.dma_start(out=cc_ho_in[ib * 128:(ib + 1) * 128, :],
                          in_=h_out_bf[:, ib, :])
    houtT = sc.tile([32, 4, 128], f32)
    for ib in range(4):
        ps_t2 = psC.tile([32, 128], f32, tag="tr2", bufs=2)
        nc.tensor.transpose(ps_t2, h_out_sb[:, ib, :], ident128)
        nc.scalar.copy(out=houtT[:, ib, :], in_=ps_t2)
    ps_s2 = psC.tile([2, R], f32)
    nc.tensor.matmul(ps_s2, lhsT=ao_sb, rhs=houtT.rearrange("p a b -> p (a b)"),
                     start=True, stop=True)
    s2_sb = sc.tile([2, R], f32)
    nc.vector.tensor_copy(out=s2_sb, in_=ps_s2)
    nc.sync.dma_start(out=cc_s2_in, in_=s2_sb)

    nc.gpsimd.collective_compute(AG, mybir.AluOpType.bypass, replica_groups=groups,
                                 ins=[cc_s2_in[:]], outs=[cc_s2_out[:]])
    nc.gpsimd.collective_compute(AG, mybir.AluOpType.bypass, replica_groups=groups,
                                 ins=[cc_ho_in[:]], outs=[cc_ho_out[:]])

    row2 = cc_s2_in[0:1, :]
    bc2 = bass.AP(tensor=row2.tensor, offset=row2.offset, ap=[[0, 128]] + row2.ap[1:])
    src2f = sc.tile([128, R], f32)
    nc.sync.dma_start(out=src2f, in_=bc2)
    nc.vector.tensor_copy(out=src2_bc, in_=src2f)
    nc.vector.tensor_scalar(out=src202_bc, in0=src2f, scalar1=ALPHA,
                            scalar2=None, op0=OP.mult)
    for core in range(NCORES):
        nc.sync.dma_start(
            out=s2dst[:, core * 4:(core + 1) * 4],
            in_=cc_s2_out[core, 1, :].rearrange("(l p) -> p l", p=128))
    nc.vector.tensor_scalar(out=s2dst02, in0=s2dst, scalar1=ALPHA,
                            scalar2=None, op0=OP.mult)
    for core in range(NCORES):
        nc.gpsimd.dma_start(
            out=h2_rhs[:, core * 4:(core + 1) * 4, 0:NCLASS],
            in_=cc_ho_out[core, :, :].rearrange("(l p) c -> p l c", p=128))

    psC_ctx.close()

    # =================== Stage D: layer-2 attention + log_softmax ==========
    stD = ExitStack()
    sd = stD.enter_context(tc.tile_pool(name="sd", bufs=1))
    psD = stD.enter_context(tc.tile_pool(name="psD", bufs=1, space="PSUM"))

    ps_o2 = [psD.tile([128, NCLASS + 1], f32, name=f"ps_o2_{ib}") for ib in range(4)]

    def sink2(jb, qs):
        for ib in range(4):
            nc.tensor.matmul(ps_o2[ib], lhsT=qs[:, ib * 128:(ib + 1) * 128],
                             rhs=h2_rhs[:, jb, :],
                             start=(jb == 0), stop=(jb == NJB - 1))

    attend_tiles(src2_bc, src202_bc, lambda jb: s2dst[:, jb:jb + 1],
                 lambda jb: s2dst02[:, jb:jb + 1], sink2, goff=64)

    for ib in range(4):
        dinv2 = sd.tile([128, 1], f32, tag="dinv2", bufs=2)
        nc.vector.reciprocal(out=dinv2, in_=ps_o2[ib][:, NCLASS:NCLASS + 1])
        o2 = sd.tile([128, NCLASS], f32, tag="o2", bufs=2)
        nc.vector.tensor_scalar(out=o2, in0=ps_o2[ib][:, 0:NCLASS], scalar1=dinv2,
                                scalar2=None, op0=OP.mult)
        mx = sd.tile([128, 1], f32, tag="mx", bufs=2)
        nc.vector.tensor_reduce(out=mx, in_=o2, axis=mybir.AxisListType.X, op=OP.max)
        negmx = sd.tile([128, 1], f32, tag="negmx", bufs=2)
        nc.vector.tensor_scalar(out=negmx, in0=mx, scalar1=-1.0, scalar2=None,
                                op0=OP.mult)
        eo = sd.tile([128, NCLASS], f32, tag="eo", bufs=2)
        nc.scalar.activation(out=eo, in_=o2, func=AF.Exp, bias=negmx)
        se = sd.tile([128, 1], f32, tag="se", bufs=2)
        nc.vector.tensor_reduce(out=se, in_=eo, axis=mybir.AxisListType.X, op=OP.add)
        lse = sd.tile([128, 1], f32, tag="lse", bufs=2)
        nc.scalar.activation(out=lse, in_=se, func=AF.Ln)
        b2 = sd.tile([128, 1], f32, tag="b2", bufs=2)
        nc.vector.tensor_tensor(out=b2, in0=mx, in1=lse, op=OP.add)
        res = sd.tile([128, NCLASS], f32, tag="res", bufs=2)
        nc.vector.tensor_scalar(out=res, in0=o2, scalar1=b2, scalar2=None,
                                op0=OP.subtract)
        nc.sync.dma_start(out=out[ib * 128:(ib + 1) * 128, :], in_=res)

    stD.close()
    stC.close()
    stB.close()
    cst_ctx.close()


def _prep_inputs(x, adj, W_heads, b_heads, a_heads, W_out, b_out, a_out):
    """Host-side layout prep (slicing/transpose/dtype only)."""
    x = np.asarray(x, dtype=np.float32)
    adj = np.asarray(adj)
    W_heads = np.asarray(W_heads, dtype=np.float32)
    b_heads = np.asarray(b_heads, dtype=np.float32)
    a_heads = np.asarray(a_heads, dtype=np.float32)
    W_out = np.asarray(W_out, dtype=np.float32)
    b_out = np.asarray(b_out, dtype=np.float32)
    a_out = np.asarray(a_out, dtype=np.float32)

    wh = np.concatenate([W_heads, b_heads[:, None, :]], axis=1).astype(BF)
    aT = np.stack([a_heads[:, :NHID], a_heads[:, NHID:]], axis=2)  # [8, 64, 2]
    aT = np.ascontiguousarray(aT).astype(BF)
    wo = np.concatenate([W_out, b_out[None, :]], axis=0).astype(BF)  # [513, 32]
    ao = np.stack([a_out[:NCLASS], a_out[NCLASS:]], axis=1)  # [32, 2]
    ao = np.ascontiguousarray(ao)

    in_maps = []
    for c in range(NCORES):
        rs = slice(c * R, (c + 1) * R)
        xTc = np.concatenate([np.ascontiguousarray(x[rs].T),
                              np.ones((1, R), np.float32)], axis=0).astype(BF)
        adjTc = np.ascontiguousarray(adj[rs].T).astype(BF)
        in_maps.append({"xT": xTc, "wh": wh, "adjT": adjTc, "aT": aT,
                        "wo": wo, "ao": ao})
    return in_maps


def kernel(**inputs) -> np.ndarray:
    if "nc" not in _cached:
        _cached["nc"] = _build_program()
    nc = _cached["nc"]
    in_maps = _prep_inputs(**inputs)
    last_err = None
    for _attempt in range(3):
        try:
            res = run_bass_kernel_spmd(nc, in_maps, list(range(NCORES)))
            return np.concatenate([res.results[c]["out"] for c in range(NCORES)],
                                  axis=0)
        except Exception as e:  # transient device errors: retry
            last_err = e
            time.sleep(2)
    raise last_err


# revision 16
# speedup vs baseline: 1.0007x; 1.0007x over previous
"""GAT (2-layer graph attention network) on 8 Trainium2 NeuronCores.

Strategy (per spec sharding hint): shard the node dim N=4096 across 8 cores
(512 rows each). Each core computes its [512, 4096] slice of each attention
matrix; row-wise softmax is local. h (layer-1 features) and h_out (layer-2
input) are all-gathered across cores via AllGather collectives.

Key kernel structure (per core), all in "transposed" layout [j-partition,
i-free] so the att @ h contraction lands on the PE partition dim:

  e[i,j] = leakyrelu(s_src[i] + s_dst[j]) is rank-1 before the nonlinearity:
  s_src/s_dst are tiny per-node scalars, so no QK matmul is needed.
  The mask is folded additively pre-lrelu: t = s_src + s_dst + (adj-1)*1500
  => exp(lrelu(t)) == adj * exp(lrelu(e)) exactly in f32 (underflow to 0).

  Per (head, j-block) tile [128j, 512i]:
    STT-1 (DVE scalar_tensor_tensor): t = (src_bcast + s_dst[jb]) + mask_bias
    lrelu: DVE STT (t*0.2 max t) for 3/4 tiles, ACT Prelu for 1/4 (balance)
    ACT Exp (batched over 4 jb)  -> p tile (bf16)
    PE matmul: psum[65, 512] += [h[jb] | ones].T @ p   (ones row = softmax
    denominator, accumulated over all 32 j-blocks)

  Then per head: normalize by the denominator row, ELU, assemble x_catT;
  h_out = x_catT @ W_out; AllGather h_out; layer-2 attention (same scheme,
  row-form output) and log_softmax.
"""
import sys
import time

sys.path.insert(0, "/opt/trn_rl_repo")

import numpy as np
import ml_dtypes

import concourse.bass as bass
import concourse.bacc as bacc
import concourse.tile as tile
from concourse import mybir
from concourse.bass_utils import run_bass_kernel_spmd
from concourse.masks import make_identity

dt = mybir.dt
BF = ml_dtypes.bfloat16

N, NFEAT, NHID, NHEAD, NCLASS = 4096, 1024, 64, 8, 32
NCORES = 8
R = N // NCORES          # 512 rows per core
NJB = N // 128           # 32 j-blocks
KCH = NFEAT // 128       # 8 full K chunks for x@W (+1 for bias row)
MASK_BIG = 1500.0
ALPHA = 0.2

_cached = {}


def _build_program():
    nc = bacc.Bacc("TRN2", target_bir_lowering=False, debug=False,
                   enable_asserts=False, num_devices=NCORES)

    xT = nc.dram_tensor("xT", [NFEAT + 1, R], dt.bfloat16, kind="ExternalInput").ap()
    wh = nc.dram_tensor("wh", [NHEAD, NFEAT + 1, NHID], dt.bfloat16, kind="ExternalInput").ap()
    adjT = nc.dram_tensor("adjT", [N, R], dt.bfloat16, kind="ExternalInput").ap()
    aT = nc.dram_tensor("aT", [NHEAD, NHID, 2], dt.bfloat16, kind="ExternalInput").ap()
    wo = nc.dram_tensor("wo", [NHEAD * NHID + 1, NCLASS], dt.bfloat16, kind="ExternalInput").ap()
    ao = nc.dram_tensor("ao", [NCLASS, 2], dt.float32, kind="ExternalInput").ap()
    out = nc.dram_tensor("out", [R, NCLASS], dt.float32, kind="ExternalOutput").ap()

    with tile.TileContext(nc, num_cores=NCORES) as tc:
        _emit(nc, tc, xT, wh, adjT, aT, wo, ao, out)
    nc.compile()
    return nc


def _emit(nc, tc, xT, wh, adjT, aT, wo, ao, out):
    from contextlib import ExitStack
    f32, bf16 = dt.float32, dt.bfloat16
    AF = mybir.ActivationFunctionType
    OP = mybir.AluOpType
    AG = "AllGather"

    cst_ctx = ExitStack()
    cst = cst_ctx.enter_context(tc.tile_pool(name="cst", bufs=1))
    dram = cst_ctx.enter_context(tc.tile_pool(name="dram", bufs=1, space="DRAM"))

    # ---- collective buffers ----
    cc_s_in = dram.tile([2 * NHEAD, R], f32)
    cc_s_out = dram.tile([NCORES, 2 * NHEAD, R], f32, addr_space="Shared")
    cc_h_in = [dram.tile([R, NHID], bf16, name=f"cc_h_in{h}") for h in range(NHEAD)]
    cc_h_out = [dram.tile([NCORES, R, NHID], bf16, addr_space="Shared",
                          name=f"cc_h_out{h}") for h in range(NHEAD)]
    cc_ho_in = dram.tile([R, NCLASS], bf16)
    cc_ho_out = dram.tile([NCORES, R, NCLASS], bf16, addr_space="Shared")
    cc_s2_in = dram.tile([2, R], f32)
    cc_s2_out = dram.tile([NCORES, 2, R], f32, addr_space="Shared")
    groups = [list(range(NCORES))]

    # ---- persistent SBUF ----
    mT = cst.tile([128, NJB, R], bf16)            # raw 0/1 mask, transposed
    nc.scalar.dma_start(
        out=mT,
        in_=adjT.rearrange("(jb p) i -> p jb i", p=128))

    h_rhs = [cst.tile([128, NJB, NHID + 1], bf16, name=f"h_rhs{h}")
             for h in range(NHEAD)]
    for h in range(NHEAD):
        nc.vector.memset(h_rhs[h][:, :, NHID:NHID + 1], 1.0)

    src_bc = [cst.tile([128, R], bf16, name=f"src_bc{h}") for h in range(NHEAD)]
    src02_bc = [cst.tile([128, R], bf16, name=f"src02_bc{h}") for h in range(NHEAD)]
    sdst = cst.tile([128, NHEAD, NJB], f32)
    sdst02 = cst.tile([128, NHEAD, NJB], f32)
    ident64 = cst.tile([64, 64], bf16)
    make_identity(nc, ident64)
    ident128 = cst.tile([128, 128], f32)
    make_identity(nc, ident128)
    ones64 = cst.tile([1, 64], f32)
    nc.vector.memset(ones64, 1.0)
    ones_row = cst.tile([1, R], bf16)
    nc.vector.memset(ones_row, 1.0)
    xcatT = [cst.tile([128, R], bf16, name=f"xcatT{k}") for k in range(4)]
    h2_rhs = cst.tile([128, NJB, NCLASS + 1], bf16)
    nc.vector.memset(h2_rhs[:, :, NCLASS:NCLASS + 1], 1.0)
    src2_bc = cst.tile([128, R], bf16)
    src202_bc = cst.tile([128, R], bf16)
    s2dst = cst.tile([128, NJB], f32)
    s2dst02 = cst.tile([128, NJB], f32)

    # =================== Stage A: h = x @ W per head, s vectors ============
    stA = ExitStack()
    sa = stA.enter_context(tc.tile_pool(name="sa", bufs=1))
    psA = stA.enter_context(tc.tile_pool(name="psA", bufs=1, space="PSUM"))

    xT_sb = sa.tile([128, KCH + 1, R], bf16)
    nc.sync.dma_start(out=xT_sb[:, 0:KCH, :],
                      in_=xT[0:NFEAT, :].rearrange("(k p) i -> p k i", p=128))
    nc.sync.dma_start(out=xT_sb[0:1, KCH, :], in_=xT[NFEAT:NFEAT + 1, :])
    wh_sb = sa.tile([128, NHEAD, KCH + 1, NHID], bf16)
    for h in range(NHEAD):
        nc.scalar.dma_start(out=wh_sb[:, h, 0:KCH, :],
                            in_=wh[h, 0:NFEAT, :].rearrange("(k p) o -> p k o", p=128))
        nc.scalar.dma_start(out=wh_sb[0:1, h, KCH, :], in_=wh[h, NFEAT:NFEAT + 1, :])
    aT_sb = sa.tile([64, NHEAD, 2], bf16)
    nc.sync.dma_start(out=aT_sb, in_=aT.rearrange("h o k -> o h k"))

    hT_sb = sa.tile([64, NHEAD, R], bf16)
    for h in range(NHEAD):
        ps_hT = psA.tile([64, R], f32, tag="hT", bufs=2)
        for k in range(KCH + 1):
            kp = 128 if k < KCH else 1
            nc.tensor.matmul(ps_hT, lhsT=wh_sb[0:kp, h, k, :],
                             rhs=xT_sb[0:kp, k, :],
                             start=(k == 0), stop=(k == KCH))
        nc.scalar.copy(out=hT_sb[:, h, :], in_=ps_hT)
        ps_s1 = psA.tile([2, R], f32, tag="s1", bufs=2)
        nc.tensor.matmul(ps_s1, lhsT=aT_sb[:, h, :], rhs=hT_sb[:, h, :],
                         start=True, stop=True)
        s1_sb = sa.tile([2, R], f32, tag="s1sb", bufs=2)
        nc.vector.tensor_copy(out=s1_sb, in_=ps_s1)
        nc.sync.dma_start(out=cc_s_in[2 * h:2 * h + 2, :], in_=s1_sb)
        h_row4 = sa.tile([128, 4, 64], bf16, tag="hrow", bufs=2)
        for tb in range(4):
            ps_tr = psA.tile([128, 64], bf16, tag="tr", bufs=2)
            nc.tensor.transpose(ps_tr, hT_sb[:, h, tb * 128:(tb + 1) * 128], ident64)
            nc.vector.tensor_copy(out=h_row4[:, tb, :], in_=ps_tr)
        nc.sync.dma_start(out=cc_h_in[h].rearrange("(l p) o -> p l o", p=128),
                          in_=h_row4)

    nc.gpsimd.collective_compute(AG, mybir.AluOpType.bypass, replica_groups=groups,
                                 ins=[cc_s_in[:]], outs=[cc_s_out[:]])
    for h in range(NHEAD):
        nc.gpsimd.collective_compute(AG, mybir.AluOpType.bypass, replica_groups=groups,
                                     ins=[cc_h_in[h][:]], outs=[cc_h_out[h][:]])

    # src_bc: broadcast own s_src rows (local, no gather needed)
    for h in range(NHEAD):
        row = cc_s_in[2 * h:2 * h + 1, :]
        bc_ap = bass.AP(tensor=row.tensor, offset=row.offset,
                        ap=[[0, 128]] + row.ap[1:])
        srcf = sa.tile([128, R], f32, tag="srcf", bufs=2)
        nc.sync.dma_start(out=srcf, in_=bc_ap)
        nc.vector.tensor_copy(out=src_bc[h], in_=srcf)
        nc.vector.tensor_scalar(out=src02_bc[h], in0=srcf, scalar1=ALPHA,
                                scalar2=None, op0=OP.mult)

    # sdst tiles first (small, unblock the attend pipeline), then h_rhs
    for h in range(NHEAD):
        for core in range(NCORES):
            nc.sync.dma_start(
                out=sdst[:, h, core * 4:(core + 1) * 4],
                in_=cc_s_out[core, 2 * h + 1, :].rearrange("(l p) -> p l", p=128))
    sdst2d = sdst.rearrange("p a b -> p (a b)")
    sdst02_2d = sdst02.rearrange("p a b -> p (a b)")
    nc.vector.tensor_scalar(out=sdst02_2d, in0=sdst2d, scalar1=ALPHA,
                            scalar2=None, op0=OP.mult)
    for h in range(NHEAD):
        for core in range(NCORES):
            eng = nc.sync
            eng.dma_start(
                out=h_rhs[h][:, core * 4:(core + 1) * 4, 0:NHID],
                in_=cc_h_out[h][core, :, :].rearrange("(l p) o -> p l o", p=128))

    stA.close()

    # =================== Stage B: layer-1 attention ========================
    stB = ExitStack()
    sb_ = stB.enter_context(tc.tile_pool(name="sb", bufs=1))
    psB_ctx = ExitStack()
    psB = psB_ctx.enter_context(tc.tile_pool(name="psB", bufs=1, space="PSUM"))

    def attend_tiles(src_tile, src02_tile, sdst_ap_fn, sdst02_ap_fn, q_sink, goff):
        """Emit the 32 j-block elementwise chain; call q_sink(jb, q_slice)."""
        for jbg in range(NJB // 4):
            route_act = ((goff + jbg) % 2) == 0
            eL = sb_.tile([128, 4, R], bf16, tag="eL", bufs=3)
            if route_act:
                for j4 in range(4):
                    jb = jbg * 4 + j4
                    nc.scalar.activation(out=eL[:, j4, :], in_=src_tile,
                                         func=AF.Prelu, bias=sdst_ap_fn(jb),
                                         scale=1.0, alpha=ALPHA)
            else:
                t4 = sb_.tile([128, 4, R], bf16, tag="t4", bufs=2)
                e5 = sb_.tile([128, 4, R], bf16, tag="e5", bufs=2)
                for j4 in range(4):
                    jb = jbg * 4 + j4
                    nc.vector.tensor_scalar(out=t4[:, j4, :], in0=src_tile,
                                            scalar1=sdst_ap_fn(jb), scalar2=None,
                                            op0=OP.add)
                    nc.vector.tensor_scalar(out=e5[:, j4, :], in0=src02_tile,
                                            scalar1=sdst02_ap_fn(jb), scalar2=None,
                                            op0=OP.add)
                nc.vector.tensor_tensor(out=eL, in0=t4, in1=e5, op=OP.max)
            q = sb_.tile([128, 4, R], bf16, tag="q", bufs=3)
            nc.scalar.activation(out=q, in_=eL, func=AF.Exp)
            nc.vector.tensor_tensor(out=q, in0=q,
                                    in1=mT[:, jbg * 4:(jbg + 1) * 4, :], op=OP.mult)
            for j4 in range(4):
                q_sink(jbg * 4 + j4, q[:, j4, :])

    for h in range(NHEAD):
        ps_att = psB.tile([NHID + 1, R], f32, tag="att", bufs=2)

        def sink(jb, qs, ps_att=ps_att, h=h):
            nc.tensor.matmul(ps_att, lhsT=h_rhs[h][:, jb, :], rhs=qs,
                             start=(jb == 0), stop=(jb == NJB - 1))

        attend_tiles(src_bc[h], src02_bc[h],
                     lambda jb, h=h: sdst[:, h, jb:jb + 1],
                     lambda jb, h=h: sdst02[:, h, jb:jb + 1], sink, goff=h * 8)

        # normalize + ELU -> x_catT
        dinv = sb_.tile([1, R], f32, tag="dinv", bufs=2)
        nc.vector.reciprocal(out=dinv, in_=ps_att[NHID:NHID + 1, :])
        ps_bc = psB.tile([64, R], f32, tag="bc", bufs=2)
        nc.tensor.matmul(ps_bc, lhsT=ones64, rhs=dinv, start=True, stop=True)
        att_sb = sb_.tile([64, R], f32, tag="attsb", bufs=2)
        nc.scalar.copy(out=att_sb, in_=ps_att[0:NHID, :])
        nc.vector.tensor_tensor(out=att_sb, in0=att_sb, in1=ps_bc, op=OP.mult)
        attn = att_sb
        neg = sb_.tile([64, R], f32, tag="neg", bufs=2)
        nc.vector.tensor_scalar(out=neg, in0=attn, scalar1=0.0, scalar2=None,
                                op0=OP.min)
        q2 = sb_.tile([64, R], f32, tag="q2", bufs=2)
        nc.scalar.activation(out=q2, in_=neg, func=AF.Exp)
        pos = sb_.tile([64, R], f32, tag="pos", bufs=2)
        nc.vector.tensor_scalar(out=pos, in0=attn, scalar1=0.0, scalar2=-1.0,
                                op0=OP.max, op1=OP.add)
        nc.vector.tensor_tensor(out=xcatT[h // 2][64 * (h % 2):64 * (h % 2) + 64, :],
                                in0=pos, in1=q2, op=OP.add)

    psB_ctx.close()

    # =================== Stage C: h_out = x_cat @ W_out, s2, gathers =======
    stC = ExitStack()
    sc = stC.enter_context(tc.tile_pool(name="sc", bufs=1))
    psC_ctx = ExitStack()
    psC = psC_ctx.enter_context(tc.tile_pool(name="psC", bufs=1, space="PSUM"))

    wo_sb = sc.tile([128, 5, NCLASS], bf16)
    nc.sync.dma_start(out=wo_sb[:, 0:4, :],
                      in_=wo[0:NHEAD * NHID, :].rearrange("(k p) c -> p k c", p=128))
    nc.sync.dma_start(out=wo_sb[0:1, 4, :], in_=wo[NHEAD * NHID:NHEAD * NHID + 1, :])
    ao_sb = sc.tile([32, 2], f32)
    nc.sync.dma_start(out=ao_sb, in_=ao)

    ps_ho = psC.tile([128, 4, NCLASS], f32)
    for ib in range(4):
        isl = slice(ib * 128, (ib + 1) * 128)
        for k in range(5):
            if k < 4:
                nc.tensor.matmul(ps_ho[:, ib, :], lhsT=xcatT[k][:, isl],
                                 rhs=wo_sb[:, k, :], start=(k == 0), stop=False)
            else:
                nc.tensor.matmul(ps_ho[:, ib, :], lhsT=ones_row[:, isl],
                                 rhs=wo_sb[0:1, 4, :], start=False, stop=True)
    h_out_sb = sc.tile([128, 4, NCLASS], f32)
    nc.scalar.copy(out=h_out_sb, in_=ps_ho)
    h_out_bf = sc.tile([128, 4, NCLASS], bf16)
    nc.vector.tensor_copy(out=h_out_bf, in_=h_out_sb)
    for ib in range(4):
        nc.sync.dma_start(out=cc_ho_in[ib * 128:(ib + 1) * 128, :],
                          in_=h_out_bf[:, ib, :])
    houtT = sc.tile([32, 4, 128], f32)
    for ib in range(4):
        ps_t2 = psC.tile([32, 128], f32, tag="tr2", bufs=2)
        nc.tensor.transpose(ps_t2, h_out_sb[:, ib, :], ident128)
        nc.scalar.copy(out=houtT[:, ib, :], in_=ps_t2)
    ps_s2 = psC.tile([2, R], f32)
    nc.tensor.matmul(ps_s2, lhsT=ao_sb, rhs=houtT.rearrange("p a b -> p (a b)"),
                     start=True, stop=True)
    s2_sb = sc.tile([2, R], f32)
    nc.vector.tensor_copy(out=s2_sb, in_=ps_s2)
    nc.sync.dma_start(out=cc_s2_in, in_=s2_sb)

    nc.gpsimd.collective_compute(AG, mybir.AluOpType.bypass, replica_groups=groups,
                                 ins=[cc_s2_in[:]], outs=[cc_s2_out[:]])
    nc.gpsimd.collective_compute(AG, mybir.AluOpType.bypass, replica_groups=groups,
                                 ins=[cc_ho_in[:]], outs=[cc_ho_out[:]])

    row2 = cc_s2_in[0:1, :]
    bc2 = bass.AP(tensor=row2.tensor, offset=row2.offset, ap=[[0, 128]] + row2.ap[1:])
    src2f = sc.tile([128, R], f32)
    nc.sync.dma_start(out=src2f, in_=bc2)
    nc.vector.tensor_copy(out=src2_bc, in_=src2f)
    nc.vector.tensor_scalar(out=src202_bc, in0=src2f, scalar1=ALPHA,
                            scalar2=None, op0=OP.mult)
    for core in range(NCORES):
        nc.sync.dma_start(
            out=s2dst[:, core * 4:(core + 1) * 4],
            in_=cc_s2_out[core, 1, :].rearrange("(l p) -> p l", p=128))
    nc.vector.tensor_scalar(out=s2dst02, in0=s2dst, scalar1=ALPHA,
                            scalar2=None, op0=OP.mult)
    for core in range(NCORES):
        nc.gpsimd.dma_start(
            out=h2_rhs[:, core * 4:(core + 1) * 4, 0:NCLASS],
            in_=cc_ho_out[core, :, :].rearrange("(l p) c -> p l c", p=128))

    psC_ctx.close()

    # =================== Stage D: layer-2 attention + log_softmax ==========
    stD = ExitStack()
    sd = stD.enter_context(tc.tile_pool(name="sd", bufs=1))
    psD = stD.enter_context(tc.tile_pool(name="psD", bufs=1, space="PSUM"))

    ps_o2 = [psD.tile([128, NCLASS + 1], f32, name=f"ps_o2_{ib}") for ib in range(4)]

    def sink2(jb, qs):
        for ib in range(4):
            nc.tensor.matmul(ps_o2[ib], lhsT=qs[:, ib * 128:(ib + 1) * 128],
                             rhs=h2_rhs[:, jb, :],
                             start=(jb == 0), stop=(jb == NJB - 1))

    attend_tiles(src2_bc, src202_bc, lambda jb: s2dst[:, jb:jb + 1],
                 lambda jb: s2dst02[:, jb:jb + 1], sink2, goff=64)

    for ib in range(4):
        dinv2 = sd.tile([128, 1], f32, tag="dinv2", bufs=2)
        nc.vector.reciprocal(out=dinv2, in_=ps_o2[ib][:, NCLASS:NCLASS + 1])
        o2 = sd.tile([128, NCLASS], f32, tag="o2", bufs=2)
        nc.vector.tensor_scalar(out=o2, in0=ps_o2[ib][:, 0:NCLASS], scalar1=dinv2,
                                scalar2=None, op0=OP.mult)
        mx = sd.tile([128, 1], f32, tag="mx", bufs=2)
        nc.vector.tensor_reduce(out=mx, in_=o2, axis=mybir.AxisListType.X, op=OP.max)
        negmx = sd.tile([128, 1], f32, tag="negmx", bufs=2)
        nc.vector.tensor_scalar(out=negmx, in0=mx, scalar1=-1.0, scalar2=None,
                                op0=OP.mult)
        eo = sd.tile([128, NCLASS], f32, tag="eo", bufs=2)
        nc.scalar.activation(out=eo, in_=o2, func=AF.Exp, bias=negmx)
        se = sd.tile([128, 1], f32, tag="se", bufs=2)
        nc.vector.tensor_reduce(out=se, in_=eo, axis=mybir.AxisListType.X, op=OP.add)
        lse = sd.tile([128, 1], f32, tag="lse", bufs=2)
        nc.scalar.activation(out=lse, in_=se, func=AF.Ln)
        b2 = sd.tile([128, 1], f32, tag="b2", bufs=2)
        nc.vector.tensor_tensor(out=b2, in0=mx, in1=lse, op=OP.add)
        res = sd.tile([128, NCLASS], f32, tag="res", bufs=2)
        nc.vector.tensor_scalar(out=res, in0=o2, scalar1=b2, scalar2=None,
                                op0=OP.subtract)
        nc.sync.dma_start(out=out[ib * 128:(ib + 1) * 128, :], in_=res)

    stD.close()
    stC.close()
    stB.close()
    cst_ctx.close()


def _prep_inputs(x, adj, W_heads, b_heads, a_heads, W_out, b_out, a_out):
    """Host-side layout prep (slicing/transpose/dtype only)."""
    x = np.asarray(x, dtype=np.float32)
    adj = np.asarray(adj)
    W_heads = np.asarray(W_heads, dtype=np.float32)
    b_heads = np.asarray(b_heads, dtype=np.float32)
    a_heads = np.asarray(a_heads, dtype=np.float32)
    W_out = np.asarray(W_out, dtype=np.float32)
    b_out = np.asarray(b_out, dtype=np.float32)
    a_out = np.asarray(a_out, dtype=np.float32)

    wh = np.concatenate([W_heads, b_heads[:, None, :]], axis=1).astype(BF)
    aT = np.stack([a_heads[:, :NHID], a_heads[:, NHID:]], axis=2)  # [8, 64, 2]
    aT = np.ascontiguousarray(aT).astype(BF)
    wo = np.concatenate([W_out, b_out[None, :]], axis=0).astype(BF)  # [513, 32]
    ao = np.stack([a_out[:NCLASS], a_out[NCLASS:]], axis=1)  # [32, 2]
    ao = np.ascontiguousarray(ao)

    in_maps = []
    for c in range(NCORES):
        rs = slice(c * R, (c + 1) * R)
        xTc = np.concatenate([np.ascontiguousarray(x[rs].T),
                              np.ones((1, R), np.float32)], axis=0).astype(BF)
        adjTc = np.ascontiguousarray(adj[rs].T).astype(BF)
        in_maps.append({"xT": xTc, "wh": wh, "adjT": adjTc, "aT": aT,
                        "wo": wo, "ao": ao})
    return in_maps


def kernel(**inputs) -> np.ndarray:
    if "nc" not in _cached:
        _cached["nc"] = _build_program()
    nc = _cached["nc"]
    in_maps = _prep_inputs(**inputs)
    last_err = None
    for _attempt in range(3):
        try:
            res = run_bass_kernel_spmd(nc, in_maps, list(range(NCORES)))
            return np.concatenate([res.results[c]["out"] for c in range(NCORES)],
                                  axis=0)
        except Exception as e:  # transient device errors: retry
            last_err = e
            time.sleep(2)
    raise last_err
